# revision 1
# baseline (speedup 1.0000x reference)
"""Trainium2 Bass kernel for nn_ConvBN2d_if (ConvBN2d + integrate-and-fire SNN layer).

Reference semantics (N=32, T=10, Cin=Cout=128, H=W=32, 3x3 conv, pad 1):
  ratio  = bn_gamma / sqrt(bn_var)
  w_fold = conv_w * ratio[:,None,None,None]
  b_fold = (conv_b - bn_mean) * ratio + bn_beta
  pots[n,t] = conv(st[n,t], w_fold)                 # SNN path
  mem = b_fold; for t: mem += pots[t]; spike = mem > 1; mem -= spike
  output_features_st = spikes                       # [N,T,Cout,32,32]
  output_features_sc = relu(BN(conv(sc)))-based straight-through whose
                       forward value is exactly spike_count (up to one
                       fp32 rounding), so we return the spike count.

Strategy: data-parallel over N across 8 cores (4 samples/core). The 3x3
conv is 9 shifted matmuls on a zero-padded [128, 34*34] spike image per
(n,t), accumulated straight onto the membrane state held in PSUM.
Weights are scaled by 128 and split into fp16 hi+lo (2 matmul passes);
spike inputs are fed as 0 / (1/128) in fp16, so hi*x + lo*x accumulated
in fp32 PSUM reproduces fp32-conv numerics to ~1e-7 relative. Spikes
out as fp16 0/1 (exact); counts accumulate in fp16 (integers <= 10,
exact) and are upcast on the host.
"""
import os
import numpy as np
from contextlib import ExitStack

import concourse.bass as bass
import concourse.tile as tile
import concourse.mybir as mybir
from concourse import bacc, bass_utils

N_TOTAL, T, CIN, COUT, H, W = 32, 10, 128, 128, 32, 32
NCORES = 8
NPC = N_TOTAL // NCORES          # samples per core
HP = WP = 34                     # padded image
HW = H * W                       # 1024
NCH = 512                        # matmul free-dim chunk (1 PSUM bank)
SCALE = np.float32(128.0)        # weight scale; x carries 1/128

_cache = {}


def _build_program():
    nc = bacc.Bacc("TRN2", target_bir_lowering=False, debug=False)
    f16, f32 = mybir.dt.float16, mybir.dt.float32

    xpad_d = nc.dram_tensor("xpad", [NPC, T, CIN, HP * WP], f16, kind="ExternalInput").ap()
    whi_d = nc.dram_tensor("whi", [9, CIN, COUT], f16, kind="ExternalInput").ap()
    wlo_d = nc.dram_tensor("wlo", [9, CIN, COUT], f16, kind="ExternalInput").ap()
    bias_d = nc.dram_tensor("bias", [1, 2 * COUT], f16, kind="ExternalInput").ap()
    spk_d = nc.dram_tensor("spk", [NPC, T, COUT, HW], f16, kind="ExternalOutput").ap()
    cnt_d = nc.dram_tensor("cnt", [NPC, COUT, HW], f16, kind="ExternalOutput").ap()

    with tile.TileContext(nc) as tc, ExitStack() as ctx:
        const = ctx.enter_context(tc.tile_pool(name="const", bufs=1))
        xpool = ctx.enter_context(tc.tile_pool(name="xpool", bufs=4))
        spool = ctx.enter_context(tc.tile_pool(name="spool", bufs=6))
        cpool = ctx.enter_context(tc.tile_pool(name="cpool", bufs=2))
        mpool = ctx.enter_context(tc.tile_pool(name="mpool", bufs=2, space="PSUM"))

        whi_t = const.tile([CIN, 9 * COUT], f16)
        nc.sync.dma_start(whi_t.rearrange("p (k c) -> p k c", k=9),
                          whi_d.rearrange("k p c -> p k c"))
        wlo_t = const.tile([CIN, 9 * COUT], f16)
        nc.sync.dma_start(wlo_t.rearrange("p (k c) -> p k c", k=9),
                          wlo_d.rearrange("k p c -> p k c"))
        bias_t = const.tile([1, 2 * COUT], f16)
        nc.sync.dma_start(bias_t[:], bias_d[:])
        ones_t = const.tile([1, NCH], f16)
        nc.gpsimd.memset(ones_t[:], 1.0 / 128.0)

        for n in range(NPC):
            mem0 = mpool.tile([COUT, NCH], f32, tag="mem0")
            mem1 = mpool.tile([COUT, NCH], f32, tag="mem1")
            mems = (mem0, mem1)
            cnt_t = cpool.tile([COUT, HW], f16)
            nc.gpsimd.memset(cnt_t[:], 0.0)

            # membrane init: mem = b_fold (hi+lo), via K=1 matmuls so the
            # PSUM has_written bits are set by the PE (start=True).
            for c, m in enumerate(mems):
                nc.tensor.matmul(m[:], bias_t[0:1, 0:COUT], ones_t[:],
                                 start=True, stop=False)
                nc.tensor.matmul(m[:], bias_t[0:1, COUT:2 * COUT], ones_t[:],
                                 start=False, stop=False)

            for t in range(T):
                xp_t = xpool.tile([CIN, HP * WP], f16)
                nc.sync.dma_start(xp_t[:], xpad_d[n, t])
                xv = xp_t.rearrange("p (h w) -> p h w", w=WP)

                last_t = t == T - 1
                for p_i, w_t in enumerate((whi_t, wlo_t)):
                    for k in range(9):
                        ky, kx = divmod(k, 3)
                        lhsT = w_t[:, k * COUT:(k + 1) * COUT]
                        for c in range(2):
                            nc.tensor.matmul(
                                mems[c][:], lhsT,
                                xv[:, 16 * c + ky:16 * c + ky + 16, kx:kx + 32],
                                start=False,
                                stop=(last_t and p_i == 1 and k == 8))

                spk_t = spool.tile([COUT, HW], f16)
                for c in range(2):
                    sl = spk_t[:, NCH * c:NCH * (c + 1)]
                    nc.vector.tensor_scalar(sl, mems[c][:], 1.0, None,
                                            mybir.AluOpType.is_gt)
                    if not last_t:
                        nc.vector.tensor_tensor(mems[c][:], mems[c][:], sl,
                                                mybir.AluOpType.subtract)
                nc.gpsimd.tensor_tensor(cnt_t[:], cnt_t[:], spk_t[:],
                                        mybir.AluOpType.add)
                nc.sync.dma_start(spk_d[n, t], spk_t[:])

            nc.sync.dma_start(cnt_d[n], cnt_t[:])

    nc.compile()
    return nc


def _get_program():
    if "nc" not in _cache:
        _cache["nc"] = _build_program()
    return _cache["nc"]


def kernel(input_feature_st, input_features_sc, conv_w, conv_b,
           bn_gamma, bn_beta, bn_mean, bn_var):
    st = np.asarray(input_feature_st, dtype=np.float32)
    conv_w = np.asarray(conv_w, dtype=np.float32)
    conv_b = np.asarray(conv_b, dtype=np.float32)
    bn_gamma = np.asarray(bn_gamma, dtype=np.float32)
    bn_beta = np.asarray(bn_beta, dtype=np.float32)
    bn_mean = np.asarray(bn_mean, dtype=np.float32)
    bn_var = np.asarray(bn_var, dtype=np.float32)

    nc = _get_program()

    ratio = bn_gamma / np.sqrt(bn_var)
    w_fold = conv_w * ratio[:, None, None, None]          # [co, ci, kh, kw]
    b_fold = (conv_b - bn_mean) * ratio + bn_beta         # [co]

    # [9, ci, co] scaled weight splits
    w9 = np.ascontiguousarray(w_fold.transpose(2, 3, 1, 0).reshape(9, CIN, COUT)) * SCALE
    whi = w9.astype(np.float16)
    wlo = (w9 - whi.astype(np.float32)).astype(np.float16)
    bs = b_fold * SCALE
    bhi = bs.astype(np.float16)
    blo = (bs - bhi.astype(np.float32)).astype(np.float16)
    bias = np.concatenate([bhi, blo]).reshape(1, 2 * COUT)

    # host-side zero-pad to 34x34, value 1/128 where spiking (exact in fp16)
    xpad = np.zeros((N_TOTAL, T, CIN, HP, WP), np.float16)
    xpad[:, :, :, 1:H + 1, 1:W + 1] = (st * np.float32(1.0 / 128.0)).astype(np.float16)
    xpad = xpad.reshape(N_TOTAL, T, CIN, HP * WP)

    in_maps = [{"xpad": xpad[c * NPC:(c + 1) * NPC],
                "whi": whi, "wlo": wlo, "bias": bias} for c in range(NCORES)]
    trace = bool(int(os.environ.get("KERNEL_TRACE", "0")))
    res = bass_utils.run_bass_kernel_spmd(nc, in_maps, list(range(NCORES)),
                                          trace=trace)
    _cache["last_result"] = res

    out_st = np.empty((N_TOTAL, T, COUT, H, W), np.float32)
    out_sc = np.empty((N_TOTAL, COUT, H, W), np.float32)
    for c in range(NCORES):
        r = res.results[c]
        out_st[c * NPC:(c + 1) * NPC] = (
            r["spk"].reshape(NPC, T, COUT, H, W).astype(np.float32))
        out_sc[c * NPC:(c + 1) * NPC] = (
            r["cnt"].reshape(NPC, COUT, H, W).astype(np.float32))
    return out_st, out_sc
